# revision 1
# baseline (speedup 1.0000x reference)
"""MCR loss kernel for Trainium2 (8 NeuronCores).

Strategy:
  - Shard batch T=16 -> 2 timesteps per core (data parallel, no collectives).
  - Per core, on device: 8x8 avg-pool (as sum; the 1/64 is folded into the
    conv weights) via vector-engine strided reduces; reflect-pad + 3x3 conv
    as 3 PE matmuls with K=(dy,ic)=96; LeakyReLU(0.2); Gram G_t = V_t V_t^T
    via PE transpose + matmul, contraction over the 576 pixels.
  - Host: matrix determinant lemma
        logdet(I_576 + a V^T V) = logdet(I_96 + a V V^T)
    so only the [2,96,96] Grams leave the device; float64 Cholesky logdets
    (16 x 96x96 + 48 x 32x32, ~5 MFLOP total) finish the scalar loss.
"""

import numpy as np

_STATE = {}

# -------- fixed problem geometry (hardcoded per harness contract) --------
B, CCH, H, W = 16, 32, 192, 192
NCORES = 8
TPC = B // NCORES          # timesteps per core = 2
OUT = 24                   # pooled spatial size
PIX = OUT * OUT            # 576
M = 96                     # feature rows (3 maps x 32 channels)
ALPHA_E = 6.0              # 576 / (96 * eps)
ALPHA_C = 18.0             # 576 / (32 * eps)


DEBUG_TAPS = False


def _build_nc():
    import concourse.bass as bass
    import concourse.tile as tile
    from concourse import bacc, mybir

    DT = mybir.dt.float32
    nc = bacc.Bacc(
        "TRN2", target_bir_lowering=False, debug=False, num_devices=NCORES
    )
    if DEBUG_TAPS:
        pooled_out = nc.declare_dram_parameter(
            "pooled_out", [192, PIX], DT, isOutput=True
        )
        v_out = nc.declare_dram_parameter("v_out", [96, TPC * PIX], DT, isOutput=True)

    # x[g] for g = t*3+m : feature-map plane stacks, host-reordered
    x = nc.declare_dram_parameter("x", [TPC * 3, CCH, H, W], DT, isOutput=False)
    wt = nc.declare_dram_parameter("wt", [3, 3, 96, 32], DT, isOutput=False)
    ident = nc.declare_dram_parameter("ident", [128, 128], DT, isOutput=False)
    g_out = nc.declare_dram_parameter("g_out", [TPC, M, M], DT, isOutput=True)

    # group g = t*3+m; pass A = groups 0..3 (128 partitions), B = 4..5
    groups = [(t, m) for t in range(TPC) for m in range(3)]
    ACT = mybir.ActivationFunctionType

    with tile.TileContext(nc) as tc:
        with (
            tc.tile_pool(name="persist", bufs=1) as persist,
            tc.tile_pool(name="slabA", bufs=2) as slabA_pool,
            tc.tile_pool(name="slabB", bufs=2) as slabB_pool,
            tc.tile_pool(name="convtmp", bufs=2) as convtmp,
            tc.tile_pool(name="vtpool", bufs=3) as vtpool,
            tc.tile_pool(name="psum", bufs=2, space="PSUM") as psum_pool,
            tc.tile_pool(name="psumg", bufs=2, space="PSUM") as psumg_pool,
        ):
            wt_sb = persist.tile([96, 288], DT, tag="wt")
            nc.gpsimd.dma_start(
                out=wt_sb[:].rearrange("p (m x c) -> p m x c", m=3, x=3),
                in_=wt.ap().rearrange("m x p c -> p m x c"),
            )
            id_sb = persist.tile([128, 128], DT, tag="ident")
            nc.gpsimd.dma_start(out=id_sb[:], in_=ident.ap())

            pooledA = persist.tile([128, PIX], DT, tag="pooledA")
            pooledB = persist.tile([64, PIX], DT, tag="pooledB")
            v_sb = persist.tile([96, TPC * PIX], DT, tag="v")
            g_sb = persist.tile([96, TPC * 96], DT, tag="g")

            # ---- pooling: 4 quarter-slabs per pass; partition=(grp,ch) ----
            # quarter q covers input rows 48q..48q+47 = output rows 6q..6q+5
            for part, pool_p, pooled in (
                ("A", slabA_pool, pooledA),
                ("B", slabB_pool, pooledB),
            ):
                npart = 128 if part == "A" else 64
                glo = 0 if part == "A" else 4
                for q in range(4):
                    slab = pool_p.tile([npart, 48 * W], DT, tag=f"slab{part}")
                    rows = slice(48 * q, 48 * (q + 1))
                    nc.sync.dma_start(
                        out=slab[:],
                        in_=x.ap()[glo : glo + npart // 32, :, rows, :].rearrange(
                            "g c h w -> (g c) h w"
                        ),
                    )
                    # two-stage pool: contiguous-innermost first (DVE fast
                    # path), then the strided row reduction on the 8x-smaller
                    # intermediate
                    wsum = pool_p.tile([npart, 6 * 8 * 24], DT, tag=f"wsum{part}")
                    nc.vector.tensor_reduce(
                        out=wsum[:],
                        in_=slab[:].rearrange("p (g w) -> p g w", w=8),
                        axis=mybir.AxisListType.X,
                        op=mybir.AluOpType.add,
                    )
                    nc.vector.tensor_reduce(
                        out=pooled[:, q * 144 : (q + 1) * 144],
                        in_=wsum[:].rearrange("p (y r x) -> p y x r", r=8, x=24),
                        axis=mybir.AxisListType.X,
                        op=mybir.AluOpType.add,
                    )

            # ---- conv per group: reflect pad, 3x replicate, 3 matmuls ----
            for gi, (t, m) in enumerate(groups):
                pooled = pooledA if gi < 4 else pooledB
                po = (gi % 4) * 32 if gi < 4 else (gi - 4) * 32
                psrc = pooled[po : po + 32, :]
                p3 = psrc.rearrange("p (y x) -> p y x", y=OUT)

                xpad = convtmp.tile([32, 26 * 26], DT, tag="xpad")
                x3 = xpad[:].rearrange("p (y x) -> p y x", y=26)
                nc.gpsimd.tensor_copy(x3[:, 1:25, 1:25], p3)
                nc.gpsimd.tensor_copy(x3[:, 0:1, 1:25], p3[:, 1:2, :])
                nc.gpsimd.tensor_copy(x3[:, 25:26, 1:25], p3[:, 22:23, :])
                nc.gpsimd.tensor_copy(x3[:, :, 0:1], x3[:, :, 2:3])
                nc.gpsimd.tensor_copy(x3[:, :, 25:26], x3[:, :, 23:24])

                xrep = convtmp.tile([96, 24 * 26], DT, tag="xrep")
                for dy in range(3):
                    nc.gpsimd.tensor_copy(
                        xrep[dy * 32 : (dy + 1) * 32, :],
                        xpad[:, dy * 26 : dy * 26 + 624],
                    )
                xr3 = xrep[:].rearrange("p (y x) -> p y x", y=OUT, x=26)

                for half in range(2):
                    pc = psum_pool.tile([32, 288], DT, tag="convps")
                    for dx in range(3):
                        nc.tensor.matmul(
                            pc[:],
                            wt_sb[:, (m * 3 + dx) * 32 : (m * 3 + dx + 1) * 32],
                            xr3[:, 12 * half : 12 * half + 12, dx : dx + 24],
                            start=(dx == 0),
                            stop=(dx == 2),
                        )
                    # LeakyReLU(0.2) == max(0.2*z, z); PSUM may feed only one
                    # non-scalar input, so stage a copy through SBUF first
                    zc = convtmp.tile([32, 288], DT, tag="zcopy")
                    nc.scalar.copy(zc[:], pc[:])
                    nc.vector.scalar_tensor_tensor(
                        out=v_sb[
                            m * 32 : (m + 1) * 32,
                            t * PIX + half * 288 : t * PIX + (half + 1) * 288,
                        ],
                        in0=zc[:],
                        scalar=0.2,
                        in1=pc[:],
                        op0=mybir.AluOpType.mult,
                        op1=mybir.AluOpType.max,
                    )

            if DEBUG_TAPS:
                nc.gpsimd.dma_start(out=pooled_out[0:128], in_=pooledA[:])
                nc.gpsimd.dma_start(out=pooled_out[128:192], in_=pooledB[:])
                nc.gpsimd.dma_start(out=v_out.ap(), in_=v_sb[:])

            # ---- Gram per t: transpose V chunks, then accumulate VT^T@VT ----
            for t in range(TPC):
                gp = psumg_pool.tile([96, 96], DT, tag="gram")
                for c in range(5):
                    sz = 128 if c < 4 else 64
                    vslice = v_sb[:, t * PIX + c * 128 : t * PIX + c * 128 + sz]
                    pt = psum_pool.tile([128, 96], DT, tag="vtps")
                    nc.tensor.transpose(pt[:sz, :], vslice, id_sb[:96, :96])
                    vt = vtpool.tile([128, 96], DT, tag="vt")
                    nc.scalar.copy(vt[:sz, :], pt[:sz, :])
                    nc.tensor.matmul(
                        gp[:], vt[:sz, :], vt[:sz, :],
                        start=(c == 0), stop=(c == 4),
                    )
                nc.scalar.copy(g_sb[:, t * 96 : (t + 1) * 96], gp[:])
                nc.gpsimd.dma_start(
                    out=g_out[t], in_=g_sb[:, t * 96 : (t + 1) * 96]
                )

    nc.finalize()
    return nc


def _get_nc():
    if "nc" not in _STATE:
        _STATE["nc"] = _build_nc()
    return _STATE["nc"]


def _prep_weights(W1, W2, W3):
    # wt[m, dx, dy*32+ic, oc] = W_m[oc, ic, dy, dx] / 64   (pool-mean folded in)
    wt = np.stack(
        [np.asarray(w, np.float64).transpose(3, 2, 1, 0).reshape(3, 96, 32)
         for w in (W1, W2, W3)]
    ) / 64.0
    return np.ascontiguousarray(wt, dtype=np.float32)


def _host_loss(G):
    G = np.asarray(G, np.float64)  # [16, 96, 96]
    T = G.shape[0]
    I96 = np.eye(M)
    Me = I96[None] + ALPHA_E * G
    ld_e = 2.0 * np.log(
        np.diagonal(np.linalg.cholesky(Me), axis1=-2, axis2=-1)
    ).sum()
    blocks = np.stack(
        [G[:, 32 * c : 32 * (c + 1), 32 * c : 32 * (c + 1)] for c in range(3)]
    )  # [3, T, 32, 32]
    Mc = np.eye(32)[None, None] + ALPHA_C * blocks
    ld_c = 2.0 * np.log(
        np.diagonal(np.linalg.cholesky(Mc), axis1=-2, axis2=-1)
    ).sum()
    loss_expd = ld_e / (2.0 * T)
    loss_comp = (32.0 / M) * ld_c / (2.0 * T)
    return np.float32(loss_expd - loss_comp)


def run_device(inputs, **kw):
    """Run the bass kernel; returns (G [16,96,96], BassKernelResults)."""
    from concourse.bass_utils import run_bass_kernel_spmd

    nc = _get_nc()
    wt = _prep_weights(inputs["W1"], inputs["W2"], inputs["W3"])
    ident = np.eye(128, dtype=np.float32)
    ms = np.asarray(inputs["ms_fea"], np.float32)
    pan = np.asarray(inputs["pan_fea"], np.float32)
    alf = np.asarray(inputs["all_fea"], np.float32)
    in_maps = []
    for i in range(NCORES):
        sl = slice(TPC * i, TPC * (i + 1))
        # x[t*3+m] = (ms,pan,alf)[m][t]
        xs = np.stack([ms[sl], pan[sl], alf[sl]], axis=1).reshape(
            TPC * 3, CCH, H, W
        )
        in_maps.append(
            {"x": np.ascontiguousarray(xs), "wt": wt, "ident": ident}
        )
    res = run_bass_kernel_spmd(nc, in_maps, core_ids=list(range(NCORES)), **kw)
    G = np.concatenate([np.asarray(r["g_out"]) for r in res.results], axis=0)
    return G, res


def kernel(**inputs):
    G, _ = run_device(inputs)
    return _host_loss(G)

